# revision 1
# baseline (speedup 1.0000x reference)
"""GCNConv (SpMM + dense projection) on 8 Trainium2 NeuronCores.

out[i] = deg[i] * sum_{e in CSR row i} deg[col_e] * (X @ W)[col_e]
       = deg[i] * ( (sum_{e} deg[col_e] * X[col_e]) @ W )        (linearity)

Strategy (SPMD single program, per-core data):
  - Rows (outputs) are sharded: core c owns rows [c*12500, (c+1)*12500),
    padded to 12544 = 98 windows of 128 rows.
  - X (f32 or bf16) is replicated in each core's DRAM; edge gathers are done
    with GPSIMD dma_gather (int16 indices => X is split into 4 chunks of
    25000 rows, edges bucketed by chunk on host).
  - Edges are host-bucketed into a fixed, capacity-padded schedule of
    (super-batch of SBW windows) x (4 chunks) x (window) sub-buckets,
    capacities = max over cores, so all 8 cores run the same program and all
    data-dependence lives in tensor contents (indices / rowids / coefs).
  - Per 128-edge group: DVE builds S[e, r] = (iota == rowid_e) * deg[col_e],
    PE accumulates A^T[f, r] += G^T @ S into PSUM.  Per window: PE computes
    (A^T)^T @ W, DVE scales rows by deg[row] during PSUM evacuation.
"""

import os
import sys

sys.path.insert(0, "/opt/trn_rl_repo")

import numpy as np

N = 100000
E = 1600000
D = 128
NCORES = 8
RPC = 12500            # rows per core
NWIN = 98              # ceil(12500/128) windows per core
ROWS_PAD = NWIN * 128  # 12544
NCHUNK = 4
CHUNK = 25000

GATHER_DT = os.environ.get("GCN_GATHER_DT", "bf16")  # "f32" | "bf16"
# Max indices per dma_gather: the SWDGE descriptor ring holds 1024 descriptors
# (dynamic_dma_scratch_size/16) and single_packet=True needs the whole gather
# resident. single_packet=False lifts the cap but measured 4.7x slower
# end-to-end (per-packet SDMA overhead), so stay at 1024/single-packet.
GMAX = 1024
SBW = int(os.environ.get("GCN_SBW", "8" if GATHER_DT == "bf16" else "4"))
NSB = (NWIN + SBW - 1) // SBW
NQUEUES = int(os.environ.get("GCN_QUEUES", "1"))  # SWDGE queues for gathers

_cache = {}


def _build_schedule(degrees, row_pointers, column_index):
    """Host-side graph partitioning: per-core capacity-padded edge buckets."""
    rp = np.asarray(row_pointers, dtype=np.int64)
    ci = np.asarray(column_index, dtype=np.int64)
    deg = np.asarray(degrees, dtype=np.float32)

    row_id = np.searchsorted(rp, np.arange(E, dtype=np.int64), side="right") - 1

    NKEY = NSB * NCHUNK * SBW
    counts = np.zeros((NCORES, NKEY), dtype=np.int64)
    percore = []
    for c in range(NCORES):
        r0 = c * RPC
        es, ee = np.searchsorted(row_id, [r0, r0 + RPC])
        lr = (row_id[es:ee] - r0).astype(np.int32)
        cols = ci[es:ee].astype(np.int32)
        win = lr >> 7
        sb = win // SBW
        wl = win - sb * SBW
        chunk = cols // CHUNK
        key = (sb * NCHUNK + chunk) * SBW + wl
        order = np.argsort(key, kind="stable")
        key_s = key[order]
        counts[c] = np.bincount(key_s, minlength=NKEY)
        percore.append((lr[order], cols[order], key_s))

    cap = counts.max(axis=0)
    cap = ((cap + 127) // 128) * 128
    cap3 = cap.reshape(NSB, NCHUNK, SBW)
    # ensure every (sb, wl) window has at least one group (to zero its PSUM)
    for sb in range(NSB):
        nw = min(SBW, NWIN - sb * SBW)
        for wl in range(nw):
            if cap3[sb, :, wl].sum() == 0:
                cap3[sb, 0, wl] = 128
    cap = cap3.reshape(-1)

    slot_off = np.zeros(NKEY + 1, dtype=np.int64)
    np.cumsum(cap, out=slot_off[1:])
    totcap = int(slot_off[-1])
    ngroups_tot = totcap // 128

    idx16 = np.zeros((NCORES, 128, totcap // 16), dtype=np.int16)
    rowid = np.zeros((NCORES, 128, ngroups_tot), dtype=np.float32)
    coef = np.zeros((NCORES, 128, ngroups_tot), dtype=np.float32)
    degw = np.zeros((NCORES, 128, NWIN), dtype=np.float32)

    for c in range(NCORES):
        lr, cols, key_s = percore[c]
        bstart = np.zeros(NKEY, dtype=np.int64)
        bstart[1:] = np.cumsum(counts[c])[:-1]
        pos = np.arange(len(key_s)) - bstart[key_s]
        dest = slot_off[key_s] + pos

        idx_flat = np.zeros(totcap, dtype=np.int16)
        rid_flat = np.zeros(totcap, dtype=np.float32)
        cf_flat = np.zeros(totcap, dtype=np.float32)
        chunk_of = (key_s // SBW) % NCHUNK
        sbwl = key_s // (NCHUNK * SBW) * SBW + key_s % SBW  # global window
        idx_flat[dest] = (cols - chunk_of * CHUNK).astype(np.int16)
        rid_flat[dest] = (lr - sbwl * 128).astype(np.float32)
        cf_flat[dest] = deg[cols]

        idx16[c] = np.tile(idx_flat.reshape(-1, 16).T, (8, 1))
        rowid[c] = rid_flat.reshape(-1, 128).T
        coef[c] = cf_flat.reshape(-1, 128).T

        dpad = np.zeros(ROWS_PAD, dtype=np.float32)
        dpad[:RPC] = deg[c * RPC : (c + 1) * RPC]
        degw[c] = dpad.reshape(NWIN, 128).T

    return cap.reshape(NSB, NCHUNK, SBW), slot_off, idx16, rowid, coef, degw


def _build_bass(cap3, slot_off):
    import concourse.bacc as bacc
    import concourse.mybir as mybir
    import concourse.tile as tile

    sdt = mybir.dt.float32 if GATHER_DT == "f32" else mybir.dt.bfloat16

    totcap = int(slot_off[-1])
    gtot = totcap // 128

    nc = bacc.Bacc("TRN2", target_bir_lowering=False, num_swdge_queues=NQUEUES)
    x_d = nc.dram_tensor("x", [N, D], sdt, kind="ExternalInput")
    w_d = nc.dram_tensor("w", [D, D], mybir.dt.float32, kind="ExternalInput")
    iota_d = nc.dram_tensor("iota", [128, 128], sdt, kind="ExternalInput")
    idx_d = nc.dram_tensor("idx", [128, totcap // 16], mybir.dt.int16, kind="ExternalInput")
    rowid_d = nc.dram_tensor("rowid", [128, gtot], mybir.dt.float32, kind="ExternalInput")
    coef_d = nc.dram_tensor("coef", [128, gtot], mybir.dt.float32, kind="ExternalInput")
    degw_d = nc.dram_tensor("degw", [128, NWIN], mybir.dt.float32, kind="ExternalInput")
    out_d = nc.dram_tensor("out", [ROWS_PAD, D], mybir.dt.float32, kind="ExternalOutput")

    # max capacity per chunk tag across super-batches (for pool slot sizing)
    with tile.TileContext(nc) as tc:
        with tc.tile_pool(name="const", bufs=1) as cpool, \
             tc.tile_pool(name="gp", bufs=2) as gpool, \
             tc.tile_pool(name="sp", bufs=6) as spool, \
             tc.tile_pool(name="ep", bufs=2) as epool, \
             tc.tile_pool(name="at_ps", bufs=2, space="PSUM") as atpool, \
             tc.tile_pool(name="o_ps", bufs=2, space="PSUM") as opool:

            w_sb = cpool.tile([D, D], mybir.dt.float32, tag="w")
            nc.sync.dma_start(w_sb[:, :], w_d[:, :])
            iota_sb = cpool.tile([128, 128], sdt, tag="iota")
            nc.sync.dma_start(iota_sb[:, :], iota_d[:, :])
            degw_sb = cpool.tile([128, NWIN], mybir.dt.float32, tag="degw")
            nc.sync.dma_start(degw_sb[:, :], degw_d[:, :])
            idx_sb = cpool.tile([128, totcap // 16], mybir.dt.int16, tag="idx")
            nc.sync.dma_start(idx_sb[:, :], idx_d[:, :])
            rowid_sb = cpool.tile([128, gtot], mybir.dt.float32, tag="rowid")
            nc.sync.dma_start(rowid_sb[:, :], rowid_d[:, :])
            coef_sb = cpool.tile([128, gtot], mybir.dt.float32, tag="coef")
            nc.sync.dma_start(coef_sb[:, :], coef_d[:, :])

            gq = 0  # round-robin gather queue counter
            for sb in range(NSB):
                nw = min(SBW, NWIN - sb * SBW)
                # per-chunk gathers for this super-batch
                g_tiles = {}
                base_off = {}
                for ch in range(NCHUNK):
                    csum = int(cap3[sb, ch, :].sum())
                    if csum == 0:
                        continue
                    off = int(slot_off[(sb * NCHUNK + ch) * SBW])
                    base_off[ch] = off
                    gt = gpool.tile([128, csum // 128, D], sdt, tag=f"g{ch}")
                    for j0 in range(0, csum, GMAX):
                        n_j = min(GMAX, csum - j0)
                        nc.gpsimd.dma_gather(
                            gt[:, j0 // 128 : (j0 + n_j) // 128, :],
                            x_d[ch * CHUNK : (ch + 1) * CHUNK, :],
                            idx_sb[:, (off + j0) // 16 : (off + j0 + n_j) // 16],
                            n_j, n_j, D,
                            queue_num=gq % NQUEUES,
                        )
                        gq += 1
                    g_tiles[ch] = gt

                at_ps = atpool.tile([128, SBW * 128], mybir.dt.float32, tag="at")
                # ordered groups per window for start/stop flags
                win_groups = [[] for _ in range(nw)]
                for ch in range(NCHUNK):
                    for wl in range(nw):
                        ccap = int(cap3[sb, ch, wl])
                        goff = int(slot_off[(sb * NCHUNK + ch) * SBW + wl]) // 128
                        gbase = base_off.get(ch, 0) // 128
                        for g in range(ccap // 128):
                            win_groups[wl].append((ch, goff - gbase + g, goff + g))
                for wl in range(nw):
                    glist = win_groups[wl]
                    for i, (ch, gslot, gcol) in enumerate(glist):
                        s_sb = spool.tile([128, 128], sdt, tag="s")
                        nc.vector.tensor_scalar(
                            s_sb[:, :], iota_sb[:, :],
                            rowid_sb[:, gcol : gcol + 1],
                            coef_sb[:, gcol : gcol + 1],
                            mybir.AluOpType.is_equal, mybir.AluOpType.mult,
                        )
                        nc.tensor.matmul(
                            at_ps[:, wl * 128 : (wl + 1) * 128],
                            g_tiles[ch][:, gslot, :], s_sb[:, :],
                            start=(i == 0), stop=(i == len(glist) - 1),
                        )

                at_sb = epool.tile([128, SBW * 128], mybir.dt.float32, tag="atsb")
                nc.vector.tensor_copy(at_sb[:, : nw * 128], at_ps[:, : nw * 128])
                stage = epool.tile([128, SBW * 128], mybir.dt.float32, tag="stage")
                for wl in range(nw):
                    o_ps = opool.tile([128, 128], mybir.dt.float32, tag="o")
                    nc.tensor.matmul(
                        o_ps[:, :], at_sb[:, wl * 128 : (wl + 1) * 128],
                        w_sb[:, :], start=True, stop=True,
                    )
                    w_glob = sb * SBW + wl
                    nc.vector.tensor_scalar(
                        stage[:, wl * 128 : (wl + 1) * 128], o_ps[:, :],
                        degw_sb[:, w_glob : w_glob + 1], None,
                        mybir.AluOpType.mult,
                    )
                for wl in range(nw):
                    w_glob = sb * SBW + wl
                    nc.sync.dma_start(
                        out_d[w_glob * 128 : (w_glob + 1) * 128, :],
                        stage[:, wl * 128 : (wl + 1) * 128],
                    )

    nc.compile()
    return nc


def kernel(X, weights, degrees, row_pointers, column_index):
    from concourse.bass_utils import run_bass_kernel_spmd

    X = np.asarray(X)
    weights = np.asarray(weights, dtype=np.float32)
    degrees = np.asarray(degrees, dtype=np.float32)

    cap3, slot_off, idx16, rowid, coef, degw = _build_schedule(
        degrees, row_pointers, column_index
    )

    key = (GATHER_DT, SBW, NQUEUES, cap3.tobytes())
    if key not in _cache:
        _cache.clear()
        _cache[key] = _build_bass(cap3, slot_off)
    nc = _cache[key]

    if GATHER_DT == "f32":
        xg = X.astype(np.float32)
        iota = np.tile(np.arange(128, dtype=np.float32), (128, 1))
    else:
        import ml_dtypes

        xg = X.astype(ml_dtypes.bfloat16)
        iota = np.tile(np.arange(128, dtype=ml_dtypes.bfloat16), (128, 1))

    in_maps = []
    for c in range(NCORES):
        in_maps.append({
            "x": xg, "w": weights, "iota": iota,
            "idx": idx16[c], "rowid": rowid[c], "coef": coef[c],
            "degw": degw[c],
        })

    trace = bool(int(os.environ.get("GCN_TRACE", "0")))
    last_err = None
    for attempt in range(3):
        try:
            res = run_bass_kernel_spmd(
                nc, in_maps, core_ids=list(range(NCORES)), trace=trace
            )
            break
        except Exception as e:  # transient device-unrecoverable on cold start
            last_err = e
            import time as _time

            _time.sleep(10)
    else:
        raise last_err
    global last_results
    last_results = res

    out = np.empty((N, D), dtype=np.float32)
    for c in range(NCORES):
        out[c * RPC : (c + 1) * RPC] = res.results[c]["out"][:RPC]
    return out


last_results = None

